# revision 27
# baseline (speedup 1.0000x reference)
"""NT-Xent loss kernel for Trainium2, 8 NeuronCores.

Strategy (row-sharded similarity matrix):
  - Each core receives the full feature matrix cyclically rolled by c*1024
    rows, so every core runs the identical program: its 1024 rows are
    rolled-rows [0, 1024), its positive columns are [4096, 5120).
  - Column groups outermost (g=0..3), row tiles m=0..7, 1024-col halves
    innermost; group g+1's load/normalize/transpose overlaps group g's
    exp stream.  Four rotating [128,1024] PSUM slots hide the per-slot
    matmul->exp handoff latency that a 2x2048 layout exposes.
  - Normalization: squares on GPSIMD (DVE for the head group), segmented
    reduce on DVE; rsqrt via the magic-constant bit trick + 2 Newton
    steps on DVE (prefetched groups) or ACT ln/exp (head); scale to bf16
    on DVE.
  - PE transposes write into a just-consumed psum tile (explicit WAR dep
    on its exp) so no PSUM slot is ever held across chunks; DVE copies
    psum -> zbT.  Dummy matmuls up front keep the PE HAM clock gate warm
    (cold PE runs 1.2 GHz, warm 2.4 GHz).
  - exp(10*sim) mostly on ScalarE (activation, accum_out = row sums);
    12 of 64 half-chunks are offloaded to the DVE via the Schraudolph
    bit trick: int32(sim*A10 + BS) bitcast to f32 is a ~2%-accurate
    exp(10*sim) whose row-sum error washes out (<1e-4 in the loss).
  - Diagonal self-sim is exp(10) exactly (z normalized) -> subtract a
    constant instead of extracting it.  Positives are computed directly
    as a bf16 dot product of zbg (rows block) with zbg (positive block).
  - loss_row = ln(rowsum - e^10) - 10*pos; per-core [128, 8] tile is
    DMA'd out; the host sums partials and divides by N.
"""

import os

import numpy as np

N = 8192
D = 128
NCORES = 8
RPC = N // NCORES          # rows per core = 1024
G = 4                      # column groups
GCOLS = N // G             # 2048 columns per group
RT = RPC // 128            # row tiles per core = 8
ESC = 10.0                 # 1 / temperature
E10 = float(np.exp(10.0))  # diagonal exp value (z normalized -> sim_ii = 10)

# Schraudolph exp constants: bits(exp(10*s)) ~ int32(s*A10 + BS).
A10 = 10.0 * (2.0 ** 23) / float(np.log(2.0))   # 121022032
BS = float(np.float32(127 * 2 ** 23 - 482000))  # calibrated for zero mean err

# Chunks (g, m) computed on the DVE instead of ScalarE. g=0 must stay on
# ACT (diagonal runs through exact exp so the e^10 subtraction cancels).
DVE_CHUNKS = {(g, m) for g in (1, 2, 3) for m in (2, 5)}

_CACHE = {}
LAST_RESULTS = None


def _patch_act_tables():
    """Force Exp/Ln onto the combined natural_log_exp_and_others table set.

    The greedy table-load pass otherwise alternates between exp-only and
    ln-only sets (one ~2.7us table load per switch).  Stripping Exp/Ln from
    the competing sets leaves exactly one set that can serve them, so a
    single load covers the whole kernel.
    """
    if _CACHE.get("act_patched"):
        return
    import functools

    import concourse.bacc as bacc_mod
    import concourse.bass_interp as interp_mod
    import concourse.hw_specs as hw_specs
    import concourse.mybir as mybir

    AF = mybir.ActivationFunctionType
    orig = hw_specs.get_activation_tables

    @functools.cache
    def patched(arch):
        out = {}
        for name, funcs in orig(arch).items():
            if name != "natural_log_exp_and_others":
                funcs = funcs - {AF.Exp, AF.Ln}
            out[name] = funcs
        return out

    hw_specs.get_activation_tables = patched
    bacc_mod.get_activation_tables = patched
    interp_mod.get_activation_tables = patched
    _CACHE["act_patched"] = True


def _build():
    import concourse.mybir as mybir
    import concourse.tile as tile
    from concourse import bacc

    _patch_act_tables()

    f32 = mybir.dt.float32
    bf16 = mybir.dt.bfloat16
    i32 = mybir.dt.int32
    AX = mybir.AxisListType
    OP = mybir.AluOpType
    AF = mybir.ActivationFunctionType

    nc = bacc.Bacc(
        "TRN2",
        target_bir_lowering=False,
        debug=False,
        enable_asserts=False,
        num_devices=NCORES,
    )
    x = nc.dram_tensor("x", [N, D], f32, kind="ExternalInput").ap()
    ident_in = nc.dram_tensor("ident", [128, 128], f32, kind="ExternalInput").ap()
    out = nc.dram_tensor("loss_parts", [128, RT], f32, kind="ExternalOutput").ap()

    with tile.TileContext(nc) as tc:
        with (
            tc.tile_pool(name="const", bufs=1) as constp,
            tc.tile_pool(name="big", bufs=1) as bigp,
            tc.tile_pool(name="small", bufs=2) as smallp,
            tc.tile_pool(name="psum", bufs=1, space="PSUM") as psump,
        ):
            ident = constp.tile([128, 128], bf16, tag="ident")
            identf = constp.tile([128, 128], f32, tag="identf")
            nc.scalar.dma_start(out=identf[:], in_=ident_in)
            nc.vector.tensor_copy(ident[:], identf[:])

            # Touch Ln+Exp so the ACT table load starts early.
            warm = constp.tile([128, 1], f32, tag="warm")
            nc.vector.memset(warm[:], 1.0)
            nc.scalar.activation(warm[:], warm[:], AF.Ln)
            nc.scalar.activation(warm[:], warm[:], AF.Exp)

            eps2 = constp.tile([128, 1], f32, tag="eps2")
            nc.vector.memset(eps2[:], 1e-16)

            # Dedicated (non-rotating) tiles: lifetimes are simple and SBUF
            # is plentiful, so avoid pool-recycling hazards entirely.
            xg = [bigp.tile([128, GCOLS], f32, tag=f"xg{g}", name=f"xg{g}") for g in range(G)]
            sq = [bigp.tile([128, GCOLS], f32, tag=f"sq{g}", name=f"sq{g}") for g in range(G)]
            zbg = [bigp.tile([128, GCOLS], bf16, tag=f"zbg{g}", name=f"zbg{g}") for g in range(G)]
            zbT = [bigp.tile([128, GCOLS], bf16, tag=f"zbT{g}", name=f"zbT{g}") for g in range(G)]
            nsq = [bigp.tile([128, 16], f32, tag=f"nsq{g}", name=f"nsq{g}") for g in range(G)]
            rno = [bigp.tile([128, 16], f32, tag=f"rno{g}", name=f"rno{g}") for g in range(G)]
            # exp destinations (ACT chunks) and Schraudolph ints (DVE chunks)
            et = [bigp.tile([128, GCOLS], f32, tag=f"et{k}", name=f"et{k}") for k in range(2)]
            ei = [bigp.tile([128, GCOLS], i32, tag=f"ei{k}", name=f"ei{k}") for k in range(2)]

            racc = constp.tile([128, G * RT + 4], f32, tag="racc")   # ACT sums
            rdve = constp.tile([128, 2 * G * RT], f32, tag="rdve")       # DVE sums
            nc.vector.memset(rdve[:], 0.0)
            pos = constp.tile([128, RT], f32, tag="pos")

            def load_group(g, qs=range(4)):
                """DMA 512-row chunks of group g."""
                for q in qs:
                    src = x[g * GCOLS + q * 512 : g * GCOLS + (q + 1) * 512, :]
                    src = src.rearrange("(p s) d -> p s d", p=128)
                    dst = xg[g][:, q * 512 : (q + 1) * 512].rearrange(
                        "p (s d) -> p s d", s=4
                    )
                    eng = nc.sync if q % 2 == 0 else nc.scalar
                    eng.dma_start(out=dst, in_=src)

            def rsqrt_group(g):
                """rno = 1/sqrt(nsq) on DVE: magic-constant seed + 2 Newton
                steps (~1e-5 rel err; the eps clamp is irrelevant for randn
                data).  Keeps the rno ops off the bottleneck ScalarE."""
                ii = smallp.tile([128, 16], i32, tag="ii")
                nc.vector.tensor_scalar(
                    out=ii[:], in0=nsq[g][:].bitcast(i32),
                    scalar1=1, scalar2=None, op0=OP.logical_shift_right,
                )
                magic = smallp.tile([128, 16], i32, tag="magic")
                nc.vector.memset(magic[:], 0x5F3759DF)
                y0i = smallp.tile([128, 16], i32, tag="y0i")
                nc.vector.tensor_sub(y0i[:], magic[:], ii[:])
                y0 = y0i[:].bitcast(f32)
                t = smallp.tile([128, 16], f32, tag="t")
                for _ in range(2):
                    nc.vector.tensor_mul(t[:], y0, y0)
                    nc.vector.scalar_tensor_tensor(
                        out=t[:], in0=t[:], scalar=-0.5, in1=nsq[g][:],
                        op0=OP.mult, op1=OP.mult,
                    )
                    nc.vector.tensor_scalar_add(t[:], t[:], 1.5)
                    nc.vector.tensor_mul(rno[g][:], y0, t[:])
                    y0 = rno[g][:]

            def norm_chunk(g, q, dve, rno_act=True):
                """nsq (squares+reduce) -> rno (ACT rsqrt if rno_act)."""
                sl = slice(q * 512, (q + 1) * 512)
                eng = nc.vector if dve else nc.gpsimd
                eng.tensor_mul(sq[g][:, sl], xg[g][:, sl], xg[g][:, sl])
                nc.vector.tensor_reduce(
                    nsq[g][:, q * 4 : (q + 1) * 4],
                    sq[g][:, sl].rearrange("p (s d) -> p s d", s=4),
                    axis=AX.X, op=OP.add,
                )
                if not rno_act:
                    return
                lnv = smallp.tile([128, 4], f32, tag="lnv")
                nc.scalar.activation(lnv[:], nsq[g][:, q * 4 : (q + 1) * 4],
                                     AF.Ln, bias=eps2[:, 0:1])
                nc.scalar.activation(rno[g][:, q * 4 : (q + 1) * 4],
                                     lnv[:], AF.Exp, scale=-0.5)

            def scale_chunk(g, q, dve=True):
                """zbg = xg * rno (per 128-col block, f32 -> bf16)."""
                eng = nc.vector if dve else nc.gpsimd
                for j in range(4):
                    s = 4 * q + j
                    eng.tensor_scalar_mul(
                        zbg[g][:, s * 128 : (s + 1) * 128],
                        xg[g][:, s * 128 : (s + 1) * 128],
                        rno[g][:, s : s + 1],
                    )

            def transpose_chunk(g, q, ptr, dep=None):
                """PE-transpose 512 cols of zbg into psum, DVE-copy to zbT.

                ptr is a [128, 512] bf16 view carved out of a pt-pool tile
                that has just been consumed, so no extra PSUM slot is held.
                """
                from bass_rust import add_dep_helper
                for j in range(4):
                    s = 4 * q + j
                    tr = nc.tensor.transpose(
                        ptr[:, j * 128 : (j + 1) * 128],
                        zbg[g][:, s * 128 : (s + 1) * 128],
                        ident[:],
                    )
                    if dep is not None and j == 0:
                        add_dep_helper(tr.ins, dep.ins, sync=True,
                                       reason="transpose WAR on psum consumer")
                nc.vector.tensor_copy(
                    zbT[g][:, q * 512 : (q + 1) * 512], ptr[:]
                )

            def mm_pair(g, m, half):
                """One 2048-col chunk pair on a mega-tile half (cols
                half*2048..): 4 matmuls, then ONE 2048-wide ACT exp (paying
                the 352-cycle ACT op overhead once) or two DVE Schraudolph
                converts.  Region-level deps on the mega tile give the same
                4-deep pipelining the rotating-slot pool had."""
                base = half * 2048
                reg = PT[:, base : base + 2048]
                lhs = zbT[0][:, m * 128 : (m + 1) * 128]
                for k in range(4):
                    nc.tensor.matmul(
                        reg[:, k * 512 : (k + 1) * 512],
                        lhs,
                        zbT[g][:, k * 512 : (k + 1) * 512],
                    )
                if (g, m) in DVE_CHUNKS:
                    # Schraudolph: bits(exp(10*s)) = int32(s*A10 + BS)
                    for h in range(2):
                        cons = nc.vector.tensor_scalar(
                            out=ei[h][:, 0:1024],
                            in0=reg[:, h * 1024 : (h + 1) * 1024],
                            scalar1=A10,
                            scalar2=BS,
                            op0=OP.mult,
                            op1=OP.add,
                        )
                        nc.vector.tensor_reduce(
                            rdve[:, (g * RT + m) * 2 + h : (g * RT + m) * 2 + h + 1],
                            ei[h][:, 0:1024].bitcast(f32),
                            axis=AX.X, op=OP.add,
                        )
                else:
                    cons = nc.scalar.activation(
                        et[0][:, 0:2048], reg[:], AF.Exp, scale=ESC,
                        accum_out=racc[:, g * RT + m : g * RT + m + 1],
                    )
                return reg, cons

            # ---- head: group 0 pipelined at 512-row granularity.  Two
            # pool tiles host everything: per-q matmul strips (f32 cols
            # 0-1024), a dummy-matmul warm strip (cols 1024-1040), and the
            # transpose scratch (bf16 bytes 6-8K), so slot lifetimes end at
            # the last sub-exp and the stream rotation starts cleanly.
            # Dummy matmuls keep the PE HAM clock gate warm (cold PE runs
            # 1.2 GHz, warm 2.4 GHz).
            dumm = bigp.tile([128, 128], bf16, tag="dumm")
            nc.vector.memset(dumm[:], 0.0)
            PT = psump.tile([128, 4096], f32, tag="PT")

            def pe_warm(n):
                for _ in range(n):
                    nc.tensor.matmul(PT[:, 512:640], ident[:], dumm[:])

            pe_warm(40)
            load_group(0)
            for q in range(4):
                base = q * 1024
                norm_chunk(0, q, dve=True)
                scale_chunk(0, q)
                trv = PT.bitcast(bf16)[:, 2 * base + 1536 : 2 * base + 2048]
                transpose_chunk(0, q, trv)
                if q < 3:
                    pe_warm(8)
                nc.tensor.matmul(
                    PT[:, base : base + 512],
                    zbT[0][:, 0:128],
                    zbT[0][:, q * 512 : (q + 1) * 512],
                )
                nc.scalar.activation(
                    et[0][:, q * 512 : (q + 1) * 512],
                    PT[:, base : base + 512],
                    AF.Exp, scale=ESC,
                    accum_out=racc[:, G * RT + q : G * RT + q + 1],
                )
            load_group(1)
            # ---- main stream: g outer, m inner, 2048-col pairs on
            # alternating mega-tile halves.  Group g+1's transposes run as
            # bursts of 8 into a just-consumed pair region (explicit WAR
            # dep), so PSUM is never over-held.
            pair_i = 0
            for g in range(G):
                ms = range(1, RT) if g == 0 else range(RT)
                for m in ms:
                    reg, cons = mm_pair(g, m, pair_i % 2)
                    pair_i += 1
                    if g + 1 < G:
                        if m == (1 if g == 0 else 0):
                            for q in range(4):
                                norm_chunk(g + 1, q, dve=False, rno_act=False)
                            rsqrt_group(g + 1)
                        elif m == (2 if g == 0 else 1):
                            for q in range(4):
                                scale_chunk(g + 1, q)
                        elif m == 3:
                            trv = reg.bitcast(bf16)
                            transpose_chunk(g + 1, 0, trv[:, 0:512], dep=cons)
                            transpose_chunk(g + 1, 1, trv[:, 512:1024], dep=cons)
                        elif m == 5:
                            trv = reg.bitcast(bf16)
                            transpose_chunk(g + 1, 2, trv[:, 0:512], dep=cons)
                            transpose_chunk(g + 1, 3, trv[:, 512:1024], dep=cons)
                        elif m == 6 and g + 2 < G:
                            load_group(g + 2)
                if g == 2:
                    # positives: pos[p, s] = sum_d zbg0[p,s,d] * zbg2[p,s,d]
                    pz = bigp.tile([128, RPC], bf16, tag="pz")
                    nc.vector.tensor_mul(pz[:], zbg[0][:, 0:RPC], zbg[2][:, 0:RPC])
                    nc.vector.tensor_reduce(
                        pos[:],
                        pz[:].rearrange("p (s d) -> p s d", s=RT),
                        axis=AX.X, op=OP.add,
                    )

            # ---- epilogue: loss = ln(rowsum - e^10) - 10*pos ----
            totd32 = smallp.tile([128, G * RT], f32, tag="totd32")
            nc.vector.tensor_reduce(
                totd32[:],
                rdve[:].rearrange("p (gm h) -> p gm h", h=2),
                axis=AX.X, op=OP.add,
            )
            tot = smallp.tile([128, RT], f32, tag="tot")
            nc.vector.tensor_reduce(
                tot[:],
                racc[:, 0 : G * RT].rearrange("p (g m) -> p m g", m=RT),
                axis=AX.X, op=OP.add,
            )
            totd = smallp.tile([128, RT], f32, tag="totd")
            nc.vector.tensor_reduce(
                totd[:],
                totd32[:].rearrange("p (g m) -> p m g", m=RT),
                axis=AX.X, op=OP.add,
            )
            th = smallp.tile([128, 1], f32, tag="th")
            nc.vector.tensor_reduce(
                th[:], racc[:, G * RT : G * RT + 4], axis=AX.X, op=OP.add
            )
            # fold the head sub-chunk sums into m=0; add ACT+DVE partials
            nc.vector.tensor_add(tot[:, 0:1], tot[:, 0:1], th[:])
            nc.vector.tensor_add(tot[:], tot[:], totd[:])
            ndall = smallp.tile([128, RT], f32, tag="ndall")
            nc.vector.tensor_scalar_add(ndall[:], tot[:], -E10)
            lnd = smallp.tile([128, RT], f32, tag="lnd")
            nc.scalar.activation(lnd[:], ndall[:], AF.Ln)
            lt = smallp.tile([128, RT], f32, tag="lt")
            nc.vector.scalar_tensor_tensor(
                out=lt[:], in0=pos[:], scalar=-ESC, in1=lnd[:],
                op0=OP.mult, op1=OP.add,
            )
            nc.sync.dma_start(out=out, in_=lt[:])

    nc.compile()
    return nc


def _get_nc():
    if "nc" not in _CACHE:
        _CACHE["nc"] = _build()
    return _CACHE["nc"]


def kernel(stacked_batch: np.ndarray) -> np.ndarray:
    global LAST_RESULTS
    from concourse.bass_utils import run_bass_kernel_spmd

    nc = _get_nc()
    xf = np.ascontiguousarray(np.asarray(stacked_batch, dtype=np.float32))
    assert xf.shape == (N, D)

    ident = np.eye(128, dtype=np.float32)
    in_maps = [
        {"x": np.ascontiguousarray(np.roll(xf, -c * RPC, axis=0)), "ident": ident}
        for c in range(NCORES)
    ]
    res = run_bass_kernel_spmd(
        nc,
        in_maps,
        core_ids=list(range(NCORES)),
        trace=bool(os.environ.get("BASS_TRACE")),
    )
    LAST_RESULTS = res
    total = 0.0
    for c in range(NCORES):
        total += float(np.asarray(res.results[c]["loss_parts"], dtype=np.float64).sum())
    return np.float32(total / N)


# revision 28
# speedup vs baseline: 1.1599x; 1.1599x over previous
"""NT-Xent loss kernel for Trainium2, 8 NeuronCores.

Strategy (row-sharded similarity matrix):
  - Each core receives the full feature matrix cyclically rolled by c*1024
    rows, so every core runs the identical program: its 1024 rows are
    rolled-rows [0, 1024), its positive columns are [4096, 5120).
  - Column groups outermost (g=0..3), row tiles m=0..7, 1024-col halves
    innermost; group g+1's load/normalize/transpose overlaps group g's
    exp stream.  Four rotating [128,1024] PSUM slots hide the per-slot
    matmul->exp handoff latency that a 2x2048 layout exposes.
  - Normalization: squares on GPSIMD (DVE for the head group), segmented
    reduce on DVE; rsqrt via the magic-constant bit trick + 2 Newton
    steps on DVE (prefetched groups) or ACT ln/exp (head); scale to bf16
    on DVE.
  - PE transposes write into a just-consumed psum tile (explicit WAR dep
    on its exp) so no PSUM slot is ever held across chunks; DVE copies
    psum -> zbT.  Dummy matmuls up front keep the PE HAM clock gate warm
    (cold PE runs 1.2 GHz, warm 2.4 GHz).
  - exp(10*sim) mostly on ScalarE (activation, accum_out = row sums);
    12 of 64 half-chunks are offloaded to the DVE via the Schraudolph
    bit trick: int32(sim*A10 + BS) bitcast to f32 is a ~2%-accurate
    exp(10*sim) whose row-sum error washes out (<1e-4 in the loss).
  - Diagonal self-sim is exp(10) exactly (z normalized) -> subtract a
    constant instead of extracting it.  Positives are computed directly
    as a bf16 dot product of zbg (rows block) with zbg (positive block).
  - loss_row = ln(rowsum - e^10) - 10*pos; per-core [128, 8] tile is
    DMA'd out; the host sums partials and divides by N.
"""

import os

import numpy as np

N = 8192
D = 128
NCORES = 8
RPC = N // NCORES          # rows per core = 1024
G = 4                      # column groups
GCOLS = N // G             # 2048 columns per group
RT = RPC // 128            # row tiles per core = 8
ESC = 10.0                 # 1 / temperature
E10 = float(np.exp(10.0))  # diagonal exp value (z normalized -> sim_ii = 10)

# Schraudolph exp constants: bits(exp(10*s)) ~ int32(s*A10 + BS).
A10 = 10.0 * (2.0 ** 23) / float(np.log(2.0))   # 121022032
BS = float(np.float32(127 * 2 ** 23 - 482000))  # calibrated for zero mean err

# Chunks (g, m) computed on the DVE instead of ScalarE. g=0 must stay on
# ACT (diagonal runs through exact exp so the e^10 subtraction cancels).
DVE_CHUNKS = {(g, m, h) for g in (1, 2, 3) for m in (2, 5) for h in (0, 1)}

_CACHE = {}
LAST_RESULTS = None


def _patch_act_tables():
    """Force Exp/Ln onto the combined natural_log_exp_and_others table set.

    The greedy table-load pass otherwise alternates between exp-only and
    ln-only sets (one ~2.7us table load per switch).  Stripping Exp/Ln from
    the competing sets leaves exactly one set that can serve them, so a
    single load covers the whole kernel.
    """
    if _CACHE.get("act_patched"):
        return
    import functools

    import concourse.bacc as bacc_mod
    import concourse.bass_interp as interp_mod
    import concourse.hw_specs as hw_specs
    import concourse.mybir as mybir

    AF = mybir.ActivationFunctionType
    orig = hw_specs.get_activation_tables

    @functools.cache
    def patched(arch):
        out = {}
        for name, funcs in orig(arch).items():
            if name != "natural_log_exp_and_others":
                funcs = funcs - {AF.Exp, AF.Ln}
            out[name] = funcs
        return out

    hw_specs.get_activation_tables = patched
    bacc_mod.get_activation_tables = patched
    interp_mod.get_activation_tables = patched
    _CACHE["act_patched"] = True


def _build():
    import concourse.mybir as mybir
    import concourse.tile as tile
    from concourse import bacc

    _patch_act_tables()

    f32 = mybir.dt.float32
    bf16 = mybir.dt.bfloat16
    i32 = mybir.dt.int32
    AX = mybir.AxisListType
    OP = mybir.AluOpType
    AF = mybir.ActivationFunctionType

    nc = bacc.Bacc(
        "TRN2",
        target_bir_lowering=False,
        debug=False,
        enable_asserts=False,
        num_devices=NCORES,
    )
    x = nc.dram_tensor("x", [N, D], f32, kind="ExternalInput").ap()
    ident_in = nc.dram_tensor("ident", [128, 128], f32, kind="ExternalInput").ap()
    out = nc.dram_tensor("loss_parts", [128, RT], f32, kind="ExternalOutput").ap()

    with tile.TileContext(nc) as tc:
        with (
            tc.tile_pool(name="const", bufs=1) as constp,
            tc.tile_pool(name="big", bufs=1) as bigp,
            tc.tile_pool(name="small", bufs=2) as smallp,
            tc.tile_pool(name="psum", bufs=4, space="PSUM") as psump,
        ):
            ident = constp.tile([128, 128], bf16, tag="ident")
            identf = constp.tile([128, 128], f32, tag="identf")
            nc.scalar.dma_start(out=identf[:], in_=ident_in)
            nc.vector.tensor_copy(ident[:], identf[:])

            # Touch Ln+Exp so the ACT table load starts early.
            warm = constp.tile([128, 1], f32, tag="warm")
            nc.vector.memset(warm[:], 1.0)
            nc.scalar.activation(warm[:], warm[:], AF.Ln)
            nc.scalar.activation(warm[:], warm[:], AF.Exp)

            eps2 = constp.tile([128, 1], f32, tag="eps2")
            nc.vector.memset(eps2[:], 1e-16)

            # Dedicated (non-rotating) tiles: lifetimes are simple and SBUF
            # is plentiful, so avoid pool-recycling hazards entirely.
            xg = [bigp.tile([128, GCOLS], f32, tag=f"xg{g}", name=f"xg{g}") for g in range(G)]
            sq = [bigp.tile([128, GCOLS], f32, tag=f"sq{g}", name=f"sq{g}") for g in range(G)]
            zbg = [bigp.tile([128, GCOLS], bf16, tag=f"zbg{g}", name=f"zbg{g}") for g in range(G)]
            zbT = [bigp.tile([128, GCOLS], bf16, tag=f"zbT{g}", name=f"zbT{g}") for g in range(G)]
            nsq = [bigp.tile([128, 16], f32, tag=f"nsq{g}", name=f"nsq{g}") for g in range(G)]
            rno = [bigp.tile([128, 16], f32, tag=f"rno{g}", name=f"rno{g}") for g in range(G)]
            # exp destinations (ACT chunks) and Schraudolph ints (DVE chunks)
            et = [bigp.tile([128, GCOLS], f32, tag=f"et{k}", name=f"et{k}") for k in range(2)]
            ei = [bigp.tile([128, GCOLS], i32, tag=f"ei{k}", name=f"ei{k}") for k in range(2)]

            racc = constp.tile([128, 2 * G * RT + 4], f32, tag="racc")   # ACT sums
            rdve = constp.tile([128, 2 * G * RT], f32, tag="rdve")       # DVE sums
            nc.vector.memset(rdve[:], 0.0)
            pos = constp.tile([128, RT], f32, tag="pos")

            def load_group(g, qs=range(4)):
                """DMA 512-row chunks of group g."""
                for q in qs:
                    src = x[g * GCOLS + q * 512 : g * GCOLS + (q + 1) * 512, :]
                    src = src.rearrange("(p s) d -> p s d", p=128)
                    dst = xg[g][:, q * 512 : (q + 1) * 512].rearrange(
                        "p (s d) -> p s d", s=4
                    )
                    eng = nc.sync if q % 2 == 0 else nc.scalar
                    eng.dma_start(out=dst, in_=src)

            def rsqrt_group(g):
                """rno = 1/sqrt(nsq) on DVE: magic-constant seed + 2 Newton
                steps (~1e-5 rel err; the eps clamp is irrelevant for randn
                data).  Keeps the rno ops off the bottleneck ScalarE."""
                ii = smallp.tile([128, 16], i32, tag="ii")
                nc.vector.tensor_scalar(
                    out=ii[:], in0=nsq[g][:].bitcast(i32),
                    scalar1=1, scalar2=None, op0=OP.logical_shift_right,
                )
                magic = smallp.tile([128, 16], i32, tag="magic")
                nc.vector.memset(magic[:], 0x5F3759DF)
                y0i = smallp.tile([128, 16], i32, tag="y0i")
                nc.vector.tensor_sub(y0i[:], magic[:], ii[:])
                y0 = y0i[:].bitcast(f32)
                t = smallp.tile([128, 16], f32, tag="t")
                for _ in range(2):
                    nc.vector.tensor_mul(t[:], y0, y0)
                    nc.vector.scalar_tensor_tensor(
                        out=t[:], in0=t[:], scalar=-0.5, in1=nsq[g][:],
                        op0=OP.mult, op1=OP.mult,
                    )
                    nc.vector.tensor_scalar_add(t[:], t[:], 1.5)
                    nc.vector.tensor_mul(rno[g][:], y0, t[:])
                    y0 = rno[g][:]

            def norm_chunk(g, q, dve, rno_act=True):
                """nsq (squares+reduce) -> rno (ACT rsqrt if rno_act)."""
                sl = slice(q * 512, (q + 1) * 512)
                eng = nc.vector if dve else nc.gpsimd
                eng.tensor_mul(sq[g][:, sl], xg[g][:, sl], xg[g][:, sl])
                nc.vector.tensor_reduce(
                    nsq[g][:, q * 4 : (q + 1) * 4],
                    sq[g][:, sl].rearrange("p (s d) -> p s d", s=4),
                    axis=AX.X, op=OP.add,
                )
                if not rno_act:
                    return
                lnv = smallp.tile([128, 4], f32, tag="lnv")
                nc.scalar.activation(lnv[:], nsq[g][:, q * 4 : (q + 1) * 4],
                                     AF.Ln, bias=eps2[:, 0:1])
                nc.scalar.activation(rno[g][:, q * 4 : (q + 1) * 4],
                                     lnv[:], AF.Exp, scale=-0.5)

            def scale_chunk(g, q, dve=True):
                """zbg = xg * rno (per 128-col block, f32 -> bf16)."""
                eng = nc.vector if dve else nc.gpsimd
                for j in range(4):
                    s = 4 * q + j
                    eng.tensor_scalar_mul(
                        zbg[g][:, s * 128 : (s + 1) * 128],
                        xg[g][:, s * 128 : (s + 1) * 128],
                        rno[g][:, s : s + 1],
                    )

            def transpose_chunk(g, q, ptr, dep=None):
                """PE-transpose 512 cols of zbg into psum, DVE-copy to zbT.

                ptr is a [128, 512] bf16 view carved out of a pt-pool tile
                that has just been consumed, so no extra PSUM slot is held.
                """
                from bass_rust import add_dep_helper
                for j in range(4):
                    s = 4 * q + j
                    tr = nc.tensor.transpose(
                        ptr[:, j * 128 : (j + 1) * 128],
                        zbg[g][:, s * 128 : (s + 1) * 128],
                        ident[:],
                    )
                    if dep is not None and j == 0:
                        add_dep_helper(tr.ins, dep.ins, sync=True,
                                       reason="transpose WAR on psum consumer")
                nc.vector.tensor_copy(
                    zbT[g][:, q * 512 : (q + 1) * 512], ptr[:]
                )

            def mm_chunk(g, m, h):
                """One 1024-col half-chunk: 2 matmuls + exp (ACT or DVE)."""
                pt = psump.tile([128, 1024], f32, tag="pt")
                lhs = zbT[0][:, m * 128 : (m + 1) * 128]
                for k in range(2):
                    c = h * 1024 + k * 512
                    nc.tensor.matmul(
                        pt[:, k * 512 : (k + 1) * 512],
                        lhs,
                        zbT[g][:, c : c + 512],
                    )
                col = (g * RT + m) * 2 + h
                if (g, m, h) in DVE_CHUNKS:
                    # Schraudolph: bits(exp(10*s)) = int32(s*A10 + BS)
                    cons = nc.vector.tensor_scalar(
                        out=ei[h][:, 0:1024],
                        in0=pt[:],
                        scalar1=A10,
                        scalar2=BS,
                        op0=OP.mult,
                        op1=OP.add,
                    )
                    nc.vector.tensor_reduce(
                        rdve[:, col : col + 1],
                        ei[h][:, 0:1024].bitcast(f32),
                        axis=AX.X, op=OP.add,
                    )
                else:
                    cons = nc.scalar.activation(
                        et[h][:, 0:1024], pt[:], AF.Exp, scale=ESC,
                        accum_out=racc[:, col : col + 1],
                    )
                return pt, cons

            # ---- head: group 0 pipelined at 512-row granularity.  Two
            # pool tiles host everything: per-q matmul strips (f32 cols
            # 0-1024), a dummy-matmul warm strip (cols 1024-1040), and the
            # transpose scratch (bf16 bytes 6-8K), so slot lifetimes end at
            # the last sub-exp and the stream rotation starts cleanly.
            # Dummy matmuls keep the PE HAM clock gate warm (cold PE runs
            # 1.2 GHz, warm 2.4 GHz).
            dumm = bigp.tile([128, 128], bf16, tag="dumm")
            nc.vector.memset(dumm[:], 0.0)
            pth = [psump.tile([128, 1024], f32, tag="pt", name=f"pth{q}")
                   for q in range(4)]

            def pe_warm(t, n):
                for _ in range(n):
                    nc.tensor.matmul(t[:, 512:640], ident[:], dumm[:])

            pe_warm(pth[0], 40)
            load_group(0)
            for q in range(4):
                ph = pth[q]
                norm_chunk(0, q, dve=True)
                scale_chunk(0, q)
                trv = ph.bitcast(bf16)[:, 1536:2048]
                transpose_chunk(0, q, trv)
                if q < 3:
                    pe_warm(pth[q + 1], 8)
                nc.tensor.matmul(
                    ph[:, 0:512],
                    zbT[0][:, 0:128],
                    zbT[0][:, q * 512 : (q + 1) * 512],
                )
                nc.scalar.activation(
                    et[0][:, q * 512 : (q + 1) * 512],
                    ph[:, 0:512],
                    AF.Exp, scale=ESC,
                    accum_out=racc[:, 2 * G * RT + q : 2 * G * RT + q + 1],
                )
            load_group(1)
            # ---- main stream: g outer, m inner, 1024-col halves.  Four
            # rotating PSUM slots hide the per-slot handoff latency.  Group
            # g+1's transposes run as bursts of 8 into just-consumed psum
            # tiles, so no extra PSUM slot is ever held.
            for g in range(G):
                if g == 0:
                    mh_order = [(m, h) for m in range(1, RT) for h in range(2)]
                else:
                    mh_order = [(m, h) for m in range(RT) for h in range(2)]
                for i, (m, h) in enumerate(mh_order):
                    pt, cons = mm_chunk(g, m, h)
                    if g + 1 < G:
                        bpos = {3: 0, 9: 2} if g == 0 else {5: 0, 11: 2}
                        burst = bpos.get(i)
                        if burst is not None:
                            trv = pt.bitcast(bf16)
                            transpose_chunk(g + 1, burst, trv[:, 0:512], dep=cons)
                            transpose_chunk(g + 1, burst + 1, trv[:, 512:1024], dep=cons)
                    if g + 1 < G:
                        if i == 0:
                            for q in range(4):
                                norm_chunk(g + 1, q, dve=False, rno_act=False)
                            rsqrt_group(g + 1)
                        elif i == 2:
                            for q in range(4):
                                scale_chunk(g + 1, q)
                        elif i == 12 and g + 2 < G:
                            load_group(g + 2)
                if g == 2:
                    # positives: pos[p, s] = sum_d zbg0[p,s,d] * zbg2[p,s,d]
                    pz = bigp.tile([128, RPC], bf16, tag="pz")
                    nc.vector.tensor_mul(pz[:], zbg[0][:, 0:RPC], zbg[2][:, 0:RPC])
                    nc.vector.tensor_reduce(
                        pos[:],
                        pz[:].rearrange("p (s d) -> p s d", s=RT),
                        axis=AX.X, op=OP.add,
                    )

            # ---- epilogue: loss = ln(rowsum - e^10) - 10*pos ----
            tot32 = smallp.tile([128, G * RT], f32, tag="tot32")
            nc.vector.tensor_reduce(
                tot32[:],
                racc[:, 0 : 2 * G * RT].rearrange("p (gm h) -> p gm h", h=2),
                axis=AX.X, op=OP.add,
            )
            totd32 = smallp.tile([128, G * RT], f32, tag="totd32")
            nc.vector.tensor_reduce(
                totd32[:],
                rdve[:].rearrange("p (gm h) -> p gm h", h=2),
                axis=AX.X, op=OP.add,
            )
            tot = smallp.tile([128, RT], f32, tag="tot")
            nc.vector.tensor_reduce(
                tot[:],
                tot32[:].rearrange("p (g m) -> p m g", m=RT),
                axis=AX.X, op=OP.add,
            )
            totd = smallp.tile([128, RT], f32, tag="totd")
            nc.vector.tensor_reduce(
                totd[:],
                totd32[:].rearrange("p (g m) -> p m g", m=RT),
                axis=AX.X, op=OP.add,
            )
            th = smallp.tile([128, 1], f32, tag="th")
            nc.vector.tensor_reduce(
                th[:], racc[:, 2 * G * RT : 2 * G * RT + 4], axis=AX.X, op=OP.add
            )
            # fold the head sub-chunk sums into m=0; add ACT+DVE partials
            nc.vector.tensor_add(tot[:, 0:1], tot[:, 0:1], th[:])
            nc.vector.tensor_add(tot[:], tot[:], totd[:])
            ndall = smallp.tile([128, RT], f32, tag="ndall")
            nc.vector.tensor_scalar_add(ndall[:], tot[:], -E10)
            lnd = smallp.tile([128, RT], f32, tag="lnd")
            nc.scalar.activation(lnd[:], ndall[:], AF.Ln)
            lt = smallp.tile([128, RT], f32, tag="lt")
            nc.vector.scalar_tensor_tensor(
                out=lt[:], in0=pos[:], scalar=-ESC, in1=lnd[:],
                op0=OP.mult, op1=OP.add,
            )
            nc.sync.dma_start(out=out, in_=lt[:])

    nc.compile()
    return nc


def _get_nc():
    if "nc" not in _CACHE:
        _CACHE["nc"] = _build()
    return _CACHE["nc"]


def kernel(stacked_batch: np.ndarray) -> np.ndarray:
    global LAST_RESULTS
    from concourse.bass_utils import run_bass_kernel_spmd

    nc = _get_nc()
    xf = np.ascontiguousarray(np.asarray(stacked_batch, dtype=np.float32))
    assert xf.shape == (N, D)

    ident = np.eye(128, dtype=np.float32)
    in_maps = [
        {"x": np.ascontiguousarray(np.roll(xf, -c * RPC, axis=0)), "ident": ident}
        for c in range(NCORES)
    ]
    res = run_bass_kernel_spmd(
        nc,
        in_maps,
        core_ids=list(range(NCORES)),
        trace=bool(os.environ.get("BASS_TRACE")),
    )
    LAST_RESULTS = res
    total = 0.0
    for c in range(NCORES):
        total += float(np.asarray(res.results[c]["loss_parts"], dtype=np.float64).sum())
    return np.float32(total / N)
